# revision 5
# baseline (speedup 1.0000x reference)
"""Trainium2 Bass kernel for nn_ContinuousCritic (permutation-invariant twin critic).

Strategy: pure data parallel over 8 NeuronCores (batch 32768 -> 4096/core).

Host-side folding: the reference's _build_inp permutation-stack is affine in the
raw concatenated input x = [obs, ag, g, act] (77 dims): inp[b,p] = A_p @ x_b + c_p.
So layer 1 of each phi twin becomes, per permutation p,
    h1[b,p] = relu(x_b @ W1eff[t,p] + b1eff[t,p]),
with W1eff = A_p^T @ W1 computed once on the host. The bias rides as an extra
constant-one input row (K 77 -> 78), so no bias op is needed for layer 1.

On-device everything is feature-major ([feat_part, batch_free]) so no transposes:
    psum[m, n] += sum_k lhsT[k, m] * rhs[k, n]   (nc.tensor.matmul)
Layer 2 + perm-sum uses relu(v + b2) = max(v, -b2) + b2: a single fused DVE
scalar_tensor_tensor (op0=max, op1=add) accumulates straight from PSUM into the
perm accumulator; the leftover +6*b2 constant is folded into rho layer-1's bias
(rb1eff = rb1 + 6 * b2 @ R1) on the host.

Matmuls run in float32r (full PE rate at N=512); all activations stay fp32.
"""

import os
import numpy as np

# ---- problem constants (hardcoded per spec) --------------------------------
B = 32768
N_CORES = 8
BS = B // N_CORES          # 4096 batch per core
TILE = 512                 # batch tile (= fp32 PSUM bank)
NT = BS // TILE            # 8 batch tiles per core
KIN = 78                   # 77 raw features + constant-one row
HID = 256
NPERM = 6
DIM_BODY, DIM_OBJ, NB = 10, 15, 3

_PROG = None  # cached (nc, meta)


# ---- host-side math ---------------------------------------------------------

def _perms():
    out = []
    for i in range(NB):
        for j in range(NB):
            if i != j:
                out.append((i, j))
    return out  # [(0,1),(0,2),(1,0),(1,2),(2,0),(2,1)] - matches itertools


def _build_inp_np(obs, ag, g, act):
    """Numpy replica of reference._build_inp."""
    b = obs.shape[0]
    obs_body = obs[:, :DIM_BODY]
    obs_obj = obs[:, DIM_BODY:].reshape(b, NB, DIM_OBJ)
    onehot = np.broadcast_to(np.eye(NB, dtype=obs.dtype), (b, NB, NB))
    feats = np.concatenate([onehot, obs_obj], axis=-1)          # [b,3,18]
    ag_o = ag.reshape(b, NB, NB)                                # OBJ_IDS == reshape
    g_o = g.reshape(b, NB, NB)
    perms = _perms()
    pi = np.array([p[0] for p in perms])
    pj = np.array([p[1] for p in perms])
    body = np.broadcast_to(obs_body[:, None, :], (b, NPERM, DIM_BODY))
    actb = np.broadcast_to(act[:, None, :], (b, NPERM, act.shape[1]))
    inp = np.concatenate([
        ag_o[:, pi], ag_o[:, pj], g_o[:, pi], g_o[:, pj],
        body, feats[:, pi], feats[:, pj], actb], axis=-1)       # [b,6,62]
    return inp


def _affine_maps():
    """inp[b,p] = A[p] @ x_b + c[p] with x = concat(obs, ag, g, act) (77 dims).

    Returns A [6,62,77] (as [77,6,62] transposed view) and c [6,62], float64.
    """
    X = np.concatenate([np.eye(77, dtype=np.float64),
                        np.zeros((1, 77), dtype=np.float64)], axis=0)  # [78,77]
    obs, ag, g, act = X[:, :55], X[:, 55:64], X[:, 64:73], X[:, 73:77]
    inp = _build_inp_np(obs, ag, g, act)   # [78, 6, 62]
    c = inp[77]                            # [6, 62] constant part
    Ax = inp[:77] - c[None]                # [77, 6, 62]; Ax[k,p,f] = A[p,f,k]
    return Ax, c


def _fold_weights(inputs):
    """Host-precompute all packed device weight arrays (float32)."""
    Ax, c = _affine_maps()
    f = np.float32
    meta = {}

    w1_list = []   # 12 chunks [78, 256], index t*6+p
    for t, (w1k, b1k) in enumerate((("phi_w1a", "phi_b1a"), ("phi_w1b", "phi_b1b"))):
        W1 = np.asarray(inputs[w1k], np.float64)   # [62, 256]
        b1 = np.asarray(inputs[b1k], np.float64)   # [256]
        # W1eff[p] [77,256] = sum_f Ax[k,p,f] * W1[f,j]
        W1eff = np.einsum("kpf,fj->pkj", Ax, W1)   # [6,77,256]
        b1eff = c @ W1 + b1[None]                  # [6,256]
        for p in range(NPERM):
            w1_list.append(np.concatenate([W1eff[p], b1eff[p][None]], 0))  # [78,256]
    w1 = np.ascontiguousarray(np.stack(w1_list), f)            # [12, 78, 256]

    def pack_kxm(W):  # [256,256] -> [128, 4*128] with col ((kc*2+half)*128+m)
        return np.ascontiguousarray(
            np.asarray(W, np.float64).reshape(2, 128, 2, 128)
            .transpose(1, 0, 2, 3).reshape(128, 512), f)

    w2 = np.stack([pack_kxm(inputs["phi_w2a"]), pack_kxm(inputs["phi_w2b"])])
    r1 = np.stack([pack_kxm(inputs["rho_w1a"]), pack_kxm(inputs["rho_w1b"])])
    r2 = np.stack([np.ascontiguousarray(np.asarray(w, f).reshape(2, 128).T)
                   for w in (inputs["rho_w2a"], inputs["rho_w2b"])])  # [2,128,2]

    negb2 = np.stack([np.ascontiguousarray(-np.asarray(b, f).reshape(2, 128).T)
                      for b in (inputs["phi_b2a"], inputs["phi_b2b"])])  # [2,128,2]

    rb1_list = []
    for b2k, r1k, rb1k in (("phi_b2a", "rho_w1a", "rho_b1a"),
                           ("phi_b2b", "rho_w1b", "rho_b1b")):
        b2 = np.asarray(inputs[b2k], np.float64)
        R1 = np.asarray(inputs[r1k], np.float64)
        rb1 = np.asarray(inputs[rb1k], np.float64)
        rb1e = rb1 + NPERM * (b2 @ R1)             # fold the +6*b2 perm-sum constant
        rb1_list.append(np.ascontiguousarray(rb1e.reshape(2, 128).T, f))
    rb1 = np.stack(rb1_list)                       # [2,128,2]

    rb2 = np.array([[np.float32(inputs["rho_b2a"][0]),
                     np.float32(inputs["rho_b2b"][0])]], f)     # [1,2]

    meta.update(w1=w1, w2=np.ascontiguousarray(w2, f), r1=np.ascontiguousarray(r1, f),
                r2=np.ascontiguousarray(r2, f), negb2=np.ascontiguousarray(negb2, f),
                rb1=rb1, rb2=rb2)
    return meta


def _build_xt(inputs):
    """xT [78, B]: rows 0..76 = concat(obs, ag, g, act) transposed, row 77 = ones."""
    x = np.concatenate([inputs["obs"], inputs["ag"], inputs["g"], inputs["act"]],
                       axis=1)                     # [B, 77]
    xt = np.empty((KIN, x.shape[0]), np.float32)
    xt[:77] = np.asarray(x, np.float32).T
    xt[77] = 1.0
    return np.ascontiguousarray(xt)


def numpy_forward(inputs):
    """Folded-math forward in numpy (for validating the folding, not the device)."""
    m = _fold_weights(inputs)
    xt = _build_xt(inputs)                          # [78, B]
    qs = []
    for t in range(2):
        acc = np.zeros((256, xt.shape[1]), np.float32)
        negb2 = np.concatenate([m["negb2"][t][:, 0], m["negb2"][t][:, 1]])  # [256]
        for p in range(NPERM):
            h = np.maximum(m["w1"][t * 6 + p].T.astype(np.float32) @ xt, 0.0)
            W2 = (m["w2"][t].reshape(128, 2, 2, 128).transpose(1, 0, 2, 3)
                  .reshape(256, 256))
            v = W2.T.astype(np.float32) @ h
            acc += np.maximum(v, negb2[:, None])
        R1 = m["r1"][t].reshape(128, 2, 2, 128).transpose(1, 0, 2, 3).reshape(256, 256)
        rb1 = np.concatenate([m["rb1"][t][:, 0], m["rb1"][t][:, 1]])
        s = np.maximum(R1.T.astype(np.float32) @ acc + rb1[:, None], 0.0)
        R2 = np.concatenate([m["r2"][t][:, 0], m["r2"][t][:, 1]])
        q = R2[None, :].astype(np.float32) @ s + m["rb2"][0, t]
        qs.append(np.ascontiguousarray(q.T))        # [B,1]
    return tuple(qs)


# ---- device program ---------------------------------------------------------

def _build_program():
    import concourse.bacc as bacc
    import concourse.mybir as mybir
    import concourse.tile as tile
    from contextlib import ExitStack

    f32 = mybir.dt.float32
    f32r = mybir.dt.float32r
    RELU = mybir.ActivationFunctionType.Relu
    IDENT = mybir.ActivationFunctionType.Identity
    MAX = mybir.AluOpType.max
    ADD = mybir.AluOpType.add

    nc = bacc.Bacc("TRN2", target_bir_lowering=False, debug=False)

    xt_d = nc.dram_tensor("xt", [KIN, BS], f32r, kind="ExternalInput")
    w1_d = nc.dram_tensor("w1", [12, KIN, 256], f32r, kind="ExternalInput")
    w2_d = nc.dram_tensor("w2", [2, 128, 512], f32r, kind="ExternalInput")
    r1_d = nc.dram_tensor("r1", [2, 128, 512], f32r, kind="ExternalInput")
    r2_d = nc.dram_tensor("r2", [2, 128, 2], f32r, kind="ExternalInput")
    negb2_d = nc.dram_tensor("negb2", [2, 128, 2], f32, kind="ExternalInput")
    rb1_d = nc.dram_tensor("rb1", [2, 128, 2], f32, kind="ExternalInput")
    rb2_d = nc.dram_tensor("rb2", [1, 2], f32, kind="ExternalInput")
    q_d = [nc.dram_tensor(f"q{t}", [1, BS], f32, kind="ExternalOutput")
           for t in range(2)]

    with tile.TileContext(nc) as tc, ExitStack() as ctx:
        wpool = ctx.enter_context(tc.tile_pool(name="wpool", bufs=1))
        xpool = ctx.enter_context(tc.tile_pool(name="xpool", bufs=1))
        h1pool = ctx.enter_context(tc.tile_pool(name="h1pool", bufs=3))
        accpool = ctx.enter_context(tc.tile_pool(name="accpool", bufs=3))
        spool = ctx.enter_context(tc.tile_pool(name="spool", bufs=2))
        qpool = ctx.enter_context(tc.tile_pool(name="qpool", bufs=4))
        pspool = ctx.enter_context(tc.tile_pool(name="pspool", bufs=3, space="PSUM"))
        pqpool = ctx.enter_context(tc.tile_pool(name="pqpool", bufs=2, space="PSUM"))

        # ---- resident constants -------------------------------------------
        w1sb = []
        for tp in range(12):
            t_ = wpool.tile([KIN, 256], f32r, tag=f"w1_{tp}")
            nc.sync.dma_start(t_[:], w1_d[tp])
            w1sb.append(t_)
        w2sb, r1sb, r2sb, negb2sb, rb1sb = [], [], [], [], []
        for t in range(2):
            a = wpool.tile([128, 512], f32r, tag=f"w2_{t}")
            nc.sync.dma_start(a[:], w2_d[t])
            w2sb.append(a)
            a = wpool.tile([128, 512], f32r, tag=f"r1_{t}")
            nc.sync.dma_start(a[:], r1_d[t])
            r1sb.append(a)
            a = wpool.tile([128, 2], f32r, tag=f"r2_{t}")
            nc.sync.dma_start(a[:], r2_d[t])
            r2sb.append(a)
            a = wpool.tile([128, 2], f32, tag=f"negb2_{t}")
            nc.sync.dma_start(a[:], negb2_d[t])
            negb2sb.append(a)
            a = wpool.tile([128, 2], f32, tag=f"rb1_{t}")
            nc.sync.dma_start(a[:], rb1_d[t])
            rb1sb.append(a)
        rb2sb = wpool.tile([1, 2], f32, tag="rb2")
        nc.sync.dma_start(rb2sb[:], rb2_d[:])
        zeros = wpool.tile([128, TILE], f32, tag="zeros")
        nc.gpsimd.memset(zeros[:], 0.0)

        # ---- x shard, chunked per batch tile ------------------------------
        xtiles = []
        for i in range(NT):
            t_ = xpool.tile([KIN, TILE], f32r, tag=f"xt_{i}")
            nc.sync.dma_start(t_[:], xt_d[:, i * TILE:(i + 1) * TILE])
            xtiles.append(t_)

        # ---- main loop -----------------------------------------------------
        for i in range(NT):
            rhs_x = xtiles[i][:]
            for t in range(2):
                acc = accpool.tile([128, 2 * TILE], f32r, tag="acc")
                for p in range(NPERM):
                    # phi layer 1: two M-halves into one 2-bank psum tile
                    ps1 = pspool.tile([128, 2 * TILE], f32, tag="ps")
                    wch = w1sb[t * 6 + p]
                    for h in range(2):
                        nc.tensor.matmul(
                            ps1[:, h * TILE:(h + 1) * TILE],
                            wch[:, h * 128:(h + 1) * 128],
                            rhs_x, start=True, stop=True)
                    h1 = h1pool.tile([128, 2 * TILE], f32r, tag="h1")
                    nc.scalar.activation(h1[:], ps1[:], RELU)

                    # phi layer 2: accumulate over 2 K-chunks per half
                    ps2 = pspool.tile([128, 2 * TILE], f32, tag="ps")
                    for h in range(2):
                        for kc in range(2):
                            nc.tensor.matmul(
                                ps2[:, h * TILE:(h + 1) * TILE],
                                w2sb[t][:, (kc * 2 + h) * 128:(kc * 2 + h + 1) * 128]
                                ,
                                h1[:, kc * TILE:(kc + 1) * TILE],
                                start=(kc == 0), stop=(kc == 1))
                    # relu(v + b2) + acc  ==  max(v, -b2) + acc (+6*b2 folded to rho1)
                    for h in range(2):
                        sl = slice(h * TILE, (h + 1) * TILE)
                        nc.vector.scalar_tensor_tensor(
                            acc[:, sl], ps2[:, sl], negb2sb[t][:, h:h + 1],
                            zeros[:] if p == 0 else acc[:, sl],
                            op0=MAX, op1=ADD)

                # rho layer 1 (bias rb1eff includes the +6*b2@R1 fold)
                ps3 = pspool.tile([128, 2 * TILE], f32, tag="ps")
                for h in range(2):
                    for kc in range(2):
                        nc.tensor.matmul(
                            ps3[:, h * TILE:(h + 1) * TILE],
                            r1sb[t][:, (kc * 2 + h) * 128:(kc * 2 + h + 1) * 128]
                            ,
                            acc[:, kc * TILE:(kc + 1) * TILE],
                            start=(kc == 0), stop=(kc == 1))
                s = spool.tile([128, 2 * TILE], f32r, tag="s")
                for h in range(2):
                    sl = slice(h * TILE, (h + 1) * TILE)
                    nc.scalar.activation(s[:, sl], ps3[:, sl], RELU,
                                         bias=rb1sb[t][:, h:h + 1])

                # rho layer 2 -> q [1, TILE]
                psq = pqpool.tile([1, TILE], f32, tag="psq")
                for kc in range(2):
                    nc.tensor.matmul(
                        psq[:], r2sb[t][:, kc:kc + 1],
                        s[:, kc * TILE:(kc + 1) * TILE],
                        start=(kc == 0), stop=(kc == 1))
                qt = qpool.tile([1, TILE], f32, tag="q")
                nc.scalar.activation(qt[:], psq[:], IDENT,
                                     bias=rb2sb[:, t:t + 1])
                nc.sync.dma_start(q_d[t][:, i * TILE:(i + 1) * TILE], qt[:])

    nc.compile()
    return nc


def _get_program():
    global _PROG
    if _PROG is None:
        _PROG = _build_program()
    return _PROG


# ---- entry points -----------------------------------------------------------

def run(inputs, trace=False):
    from concourse.bass_utils import run_bass_kernel_spmd

    nc = _get_program()
    m = _fold_weights(inputs)
    xt = _build_xt(inputs)

    shared = {k: m[k] for k in ("w1", "w2", "r1", "r2", "negb2", "rb1", "rb2")}
    in_maps = []
    for c in range(N_CORES):
        im = dict(shared)
        im["xt"] = np.ascontiguousarray(xt[:, c * BS:(c + 1) * BS])
        in_maps.append(im)

    res = run_bass_kernel_spmd(nc, in_maps, list(range(N_CORES)), trace=trace)
    qs = []
    for t in range(2):
        q = np.concatenate([res.results[c][f"q{t}"] for c in range(N_CORES)],
                           axis=1)                  # [1, B]
        qs.append(np.ascontiguousarray(q.reshape(B, 1), np.float32))
    return tuple(qs), res


def kernel(**inputs):
    inputs = {k: np.asarray(v) for k, v in inputs.items()}
    assert inputs["obs"].shape == (B, 55), inputs["obs"].shape
    qs, _ = run(inputs, trace=False)
    return qs
